# revision 1
# baseline (speedup 1.0000x reference)
"""Trainium2 Bass kernel for nn_AttnBlock (GroupNorm + single-head spatial
self-attention + residual), SPMD over 8 NeuronCores.

Sharding: data-parallel over batch B=4, x2 split over query tokens
(each core handles 2048 of the 4096 spatial tokens of one batch).
The per-core input x-slab is permuted so that the core's own query tokens
come first -> the SPMD program is identical on every core (softmax/GN are
permutation-invariant over tokens).

Device algebra (weights fused on host, fp64):
  xn = GroupNorm(x) * gn_w + gn_b                      [C, N] per batch
  scoresT[j,i] = sum_c' xn[c',j] * r[c',i],  r = W1 xn + rb
      where W1 = Wk^T Wq, rb = Wk^T bq  (bk shifts all logits of a query
      equally and cancels in softmax -> dropped exactly)
  e = exp(scoresT * C^-0.5)      (no max-subtraction needed: logits ~ N(0,1))
  u[c',i] = sum_j xnT[j,c'] e[j,i];  denom[i] = sum_j e[j,i]  (ones-matmul)
  out = W2 (u * 1/denom) + b2 + x,  W2 = Wp Wv, b2 = Wp bv + bp
"""

import os
import sys

for _p in ("/opt/trn_rl_repo", "/root/.axon_site/_ro/trn_rl_repo"):
    if os.path.isdir(_p) and _p not in sys.path:
        sys.path.insert(0, _p)

import numpy as np

B, C, H, W = 4, 512, 64, 64
N = H * W            # 4096 tokens
NQ = N // 2          # 2048 query tokens per core
T = C // 128         # 4 channel tiles
JT = N // 128        # 32 key tiles
IG = NQ // 512       # 4 query groups of 512
NUM_GROUPS = 32
EPS = 1e-5
SCALE = float(C) ** -0.5

# matmul compute dtype for the heavy matmuls:
#   "bf16" - fastest (separate fast weight load); rel err ~5e-4 (default)
#   "f32r" - ~13% slower (self-loading 4-byte weights); rel err ~2e-5
#   "f32"  - exact but 4 cycles/row on the PE
MM_DTYPE = os.environ.get("BASS_MM_DTYPE", "bf16")

_PROGRAM_CACHE = {}
LAST_RESULTS = None

# walrus disables its LDWEIGHTS optimization (incl. fast-weight-load) by
# default; re-enable it for this kernel unless BASS_LDW_OPT=0.
_LDW_PATCHED = False


def _patch_ldw_opt():
    global _LDW_PATCHED
    if _LDW_PATCHED or os.environ.get("BASS_LDW_OPT", "0") == "0":
        return
    from concourse import bass_utils as _bu

    _orig = _bu.run_command

    def _patched(argv, **kw):
        argv = [
            ("--enable-ldw-opt=true" if a == "--enable-ldw-opt=false" else a)
            for a in argv
        ]
        return _orig(argv, **kw)

    _bu.run_command = _patched
    _LDW_PATCHED = True


def _build_program(mm_dtype_name: str, repeat: int = 1, denom_mm: bool = False,
                   dma_transpose: bool = False, psa: int = 2, psp: int = 2,
                   ebufs: int = 3, dual_dma: bool = True):
    _patch_ldw_opt()
    import concourse.bass as bass
    import concourse.tile as tile
    from concourse import bacc, mybir

    f32 = mybir.dt.float32
    mm_dt = {"f32": None, "f32r": mybir.dt.float32r, "bf16": mybir.dt.bfloat16}[mm_dtype_name]
    AF = mybir.ActivationFunctionType
    OP = mybir.AluOpType

    nc = bacc.Bacc("TRN2")
    mdt_early = f32 if mm_dt is None else mm_dt

    xb_d = nc.declare_dram_parameter("xb", [C, N], f32, isOutput=False)
    # x stays fp32 on the GN path: loading it bf16 saves ~6us of prologue but
    # doubles the end-to-end error (5.5e-4 -> 1.1e-3); not worth the margin.
    xdt = f32
    w1t_d = nc.declare_dram_parameter("w1t", [C, C], mdt_early, isOutput=False)
    w2t_d = nc.declare_dram_parameter("w2t", [C, C], mdt_early, isOutput=False)
    cv_d = nc.declare_dram_parameter("cvec", [128, 4, T], f32, isOutput=False)
    gi_d = nc.declare_dram_parameter("gi", [128, 8], f32, isOutput=False)
    git_d = nc.declare_dram_parameter("git", [8, 128], f32, isOutput=False)
    id_d = nc.declare_dram_parameter("ident", [128, 128], mdt_early, isOutput=False)
    on_d = nc.declare_dram_parameter("onesc", [128, 1], f32, isOutput=False)
    onr_d = nc.declare_dram_parameter("onesr", [1, 128], f32, isOutput=False)
    out_d = nc.declare_dram_parameter("out", [C, NQ], f32, isOutput=True)

    mdt = mdt_early

    def mm(out, lhsT, rhs, start, stop):
        nc.tensor.matmul(out, lhsT, rhs, start=start, stop=stop)

    with tile.TileContext(nc) as tc:
        with (
            tc.tile_pool(name="big", bufs=2) as pbig,
            tc.tile_pool(name="const", bufs=1) as pc,
            tc.tile_pool(name="stat", bufs=2) as pst,
            tc.tile_pool(name="rpool", bufs=1) as prr,
            tc.tile_pool(name="upool", bufs=1) as puu,
            tc.tile_pool(name="epool", bufs=ebufs) as pee,
            tc.tile_pool(name="iopool", bufs=2) as pio,
            tc.tile_pool(name="psA", bufs=psa, space="PSUM") as ppA,
            tc.tile_pool(name="psU", bufs=4, space="PSUM") as ppU,
            tc.tile_pool(name="psP", bufs=psp, space="PSUM") as ppP,
        ):
            # ---- constant / weight loads ----
            weng = nc.scalar if dual_dma else nc.sync
            W1T = pc.tile([128, T, C], mdt)
            weng.dma_start(out=W1T, in_=w1t_d[:].rearrange("(t p) f -> p t f", p=128))
            W2T = pc.tile([128, T, C], mdt)
            weng.dma_start(out=W2T, in_=w2t_d[:].rearrange("(t p) f -> p t f", p=128))
            GI = pc.tile([128, 8], f32)
            nc.sync.dma_start(out=GI, in_=gi_d[:])
            GIT = pc.tile([8, 128], f32)
            nc.sync.dma_start(out=GIT, in_=git_d[:])
            IDENT = pc.tile([128, 128], mdt)
            nc.sync.dma_start(out=IDENT, in_=id_d[:])
            CV = pc.tile([128, 4, T], f32)
            nc.sync.dma_start(out=CV, in_=cv_d[:])
            RB, B2, GNW, GNB = CV[:, 0, :], CV[:, 1, :], CV[:, 2, :], CV[:, 3, :]
            ONES = pc.tile([128, 1], f32)
            nc.sync.dma_start(out=ONES, in_=on_d[:])
            ONESM = pc.tile([128, 1], mdt)
            nc.vector.tensor_copy(ONESM, ONES)
            ONESR = pc.tile([1, 128], f32)
            nc.sync.dma_start(out=ONESR, in_=onr_d[:])
            SS = pc.tile([128, T, 2], f32)  # per-channel (scale, shift)

            # ---- load x + per-tile GroupNorm (stats pipeline behind DMA) ----
            # each 16-channel group lives inside one 128-channel tile, so
            # stats -> normalize proceed per tile without a global join
            X = pbig.tile([128, T, N], xdt, tag="big")
            XN = pbig.tile([128, T, N], mdt, tag="big")
            # group-0 r accumulates per tile inside the GN loop, borrowing the
            # psU banks (idle until the first attn-V matmul)
            Pr0 = [ppU.tile([128, 512], f32, tag="Pu", name=f"Pr0_{m}") for m in range(T)]
            xb_t = xb_d[:].rearrange("(t p) n -> p t n", p=128)
            epsT = pc.tile([8, 1], f32)
            nc.vector.memset(epsT, EPS)
            for t in range(T):
                for h in range(4):
                    eng = nc.scalar if (dual_dma and h % 2 == 1) else nc.sync
                    eng.dma_start(
                        out=X[:, t, 1024 * h : 1024 * (h + 1)],
                        in_=xb_t[:, t, 1024 * h : 1024 * (h + 1)],
                    )
                stats_t = pst.tile([128, 8, 6], f32, tag="stats")
                for s in range(8):
                    nc.vector.bn_stats(out=stats_t[:, s, :], in_=X[:, t, 512 * s : 512 * (s + 1)])
                mv_t = pst.tile([128, 2], f32, tag="mv")
                nc.vector.bn_aggr(out=mv_t, in_=stats_t)
                perch_t = pst.tile([128, 2], f32, tag="perch")  # (mean, E[x^2])
                nc.vector.tensor_copy(perch_t[:, 0:1], mv_t[:, 0:1])
                nc.vector.tensor_mul(perch_t[:, 1:2], mv_t[:, 0:1], mv_t[:, 0:1])
                nc.vector.tensor_add(perch_t[:, 1:2], perch_t[:, 1:2], mv_t[:, 1:2])
                GSp = ppA.tile([8, 2], f32, tag="psA", name=f"GSp{t}")
                nc.tensor.matmul(GSp, GI, perch_t, start=True, stop=True)
                GB = pst.tile([8, 2], f32, tag="GB")  # (mean_g, rstd_g)
                tmpg = pst.tile([8, 1], f32, tag="tmpg")
                nc.vector.tensor_copy(GB, GSp)
                nc.vector.tensor_mul(tmpg, GB[:, 0:1], GB[:, 0:1])
                nc.vector.tensor_sub(GB[:, 1:2], GB[:, 1:2], tmpg)  # var_g
                nc.scalar.activation(GB[:, 1:2], GB[:, 1:2], AF.Sqrt, bias=epsT)
                nc.vector.reciprocal(GB[:, 1:2], GB[:, 1:2])
                PB = ppA.tile([128, 2], f32, tag="psA", name=f"PB{t}")
                nc.tensor.matmul(PB, GIT, GB, start=True, stop=True)
                tmpc = pst.tile([128, 1], f32, tag="tmpc")
                nc.vector.tensor_mul(SS[:, t, 0:1], PB[:, 1:2], GNW[:, t : t + 1])
                nc.vector.tensor_mul(tmpc, PB[:, 0:1], SS[:, t, 0:1])
                nc.vector.tensor_sub(SS[:, t, 1:2], GNB[:, t : t + 1], tmpc)
                # xn = x*scale + shift (two halves: first unblocks attention)
                for h in range(2):
                    nc.vector.tensor_scalar(
                        out=XN[:, t, 2048 * h : 2048 * (h + 1)],
                        in0=X[:, t, 2048 * h : 2048 * (h + 1)],
                        scalar1=SS[:, t, 0:1], scalar2=SS[:, t, 1:2],
                        op0=OP.mult, op1=OP.add,
                    )
                # group-0 r chunks fill PE waits on later tiles' GN chains
                for m in range(T):
                    mm(Pr0[m], W1T[:, t, 128 * m : 128 * (m + 1)], XN[:, t, 0:512],
                       start=(t == 0), stop=(t == T - 1))

            # ---- transpose xn -> xnT (XNT reuses X's slot once X is dead) ----
            # transposes are emitted lazily, interleaved into group 0's score
            # loop (PE executes in order: a standalone transpose phase would
            # serialize ~30us before attention can start)
            XNT = pbig.tile([128, JT, C], mdt, tag="big")
            _tp_done = [0]  # j-batches emitted so far (batches of 4 j)

            def emit_transposes(upto_j):
                while _tp_done[0] * 4 < min(upto_j, JT):
                    j0 = _tp_done[0] * 4
                    for t in range(T):
                        PT = ppA.tile([128, 4, 128], mdt, tag="psA", name=f"PT{t}_{j0}")
                        for dj in range(4):
                            nc.tensor.transpose(
                                PT[:, dj, :], XN[:, t, 128 * (j0 + dj) : 128 * (j0 + dj + 1)], IDENT
                            )
                        nc.vector.tensor_copy(XNT[:, j0 : j0 + 4, 128 * t : 128 * (t + 1)], PT)
                    _tp_done[0] += 1

            # ---- attention, per query group of 512 ----
            def emit_r(g, pr_pre=None):
                isl = slice(512 * g, 512 * (g + 1))
                r_sb = prr.tile([128, T, 512], mdt, tag="r", name=f"r{_rep}_{g}")
                for m in range(T):
                    if pr_pre is not None:
                        Pr = pr_pre[m]
                    else:
                        Pr = ppA.tile([128, 512], f32, tag="psA", name=f"Pr{_rep}_{g}_{m}")
                        for t in range(T):
                            mm(Pr, W1T[:, t, 128 * m : 128 * (m + 1)], XN[:, t, isl],
                               start=(t == 0), stop=(t == T - 1))
                    nc.vector.tensor_scalar(
                        out=r_sb[:, m, :], in0=Pr, scalar1=RB[:, m : m + 1],
                        scalar2=None, op0=OP.add,
                    )
                # prefetch residual x for this group
                xrs = []
                for mo in range(T):
                    xr = pio.tile([128, 512], f32, tag="xr", name=f"xr{_rep}_{g}_{mo}", bufs=8)
                    nc.sync.dma_start(out=xr, in_=xb_d[128 * mo : 128 * (mo + 1), isl])
                    xrs.append(xr)
                return r_sb, xrs

            def emit_scores(g, j, r_sb):
                Ps = ppA.tile([128, 512], f32, tag="psA", name=f"Ps{_rep}_{g}_{j}")
                for t in range(T):
                    mm(Ps, XN[:, t, 128 * j : 128 * (j + 1)], r_sb[:, t, :],
                       start=(t == 0), stop=(t == T - 1))
                e = pee.tile([128, 512], mdt, tag="e", name=f"e{_rep}_{g}_{j}")
                nc.scalar.activation(e, Ps, AF.Exp, scale=SCALE)
                return e

            def emit_u(g, j, e, Pu, acc_e):
                for m in range(T):
                    mm(Pu[m], XNT[:, j, 128 * m : 128 * (m + 1)], e,
                       start=(j == 0), stop=(j == JT - 1))
                if denom_mm:
                    mm(acc_e, ONESM, e, start=(j == 0), stop=(j == JT - 1))
                    return
                # accumulate exp on DVE; single denominator matmul per group
                if j == 0:
                    nc.vector.tensor_copy(acc_e, e)
                else:
                    nc.vector.tensor_add(acc_e, acc_e, e)

            def emit_norm(g, Pu, acc_e):
                if denom_mm:
                    Pd = acc_e
                else:
                    # single denominator matmul over the DVE-accumulated exps
                    Pd = ppP.tile([1, 512], f32, tag="psP", name=f"Pd{_rep}_{g}")
                    mm(Pd, ONES, acc_e, start=True, stop=True)
                rec = pio.tile([1, 512], f32, tag="rec", name=f"rec{_rep}_{g}")
                nc.vector.reciprocal(rec, Pd)
                Pb = ppP.tile([128, 512], f32, tag="psP", name=f"Pb{_rep}_{g}")
                mm(Pb, ONESR, rec, start=True, stop=True)
                rbc = pio.tile([128, 512], f32, tag="rbc", name=f"rbc{_rep}_{g}")
                nc.vector.tensor_copy(rbc, Pb)
                u_sb = puu.tile([128, T, 512], mdt, tag="u", name=f"u{_rep}_{g}")
                for m in range(T):
                    nc.vector.tensor_mul(u_sb[:, m, :], Pu[m], rbc)
                return u_sb

            def emit_proj(g, u_sb, xrs):
                isl = slice(512 * g, 512 * (g + 1))
                for mo in range(T):
                    Pp = ppP.tile([128, 512], f32, tag="psP", name=f"Pp{_rep}_{g}_{mo}")
                    for t in range(T):
                        mm(Pp, W2T[:, t, 128 * mo : 128 * (mo + 1)], u_sb[:, t, :],
                           start=(t == 0), stop=(t == T - 1))
                    o = pio.tile([128, 512], f32, tag="o", name=f"o{g}_{mo}", bufs=4)
                    nc.vector.scalar_tensor_tensor(
                        out=o, in0=Pp, scalar=B2[:, mo : mo + 1], in1=xrs[mo],
                        op0=OP.add, op1=OP.add,
                    )
                    nc.sync.dma_start(out=out_d[128 * mo : 128 * (mo + 1), isl], in_=o)

            _rep = -1
            r_sb, xrs = emit_r(0, pr_pre=Pr0)
            for _rep in range(repeat):
              for g in range(IG):
                  Pu = [ppU.tile([128, 512], f32, tag="Pu", name=f"Pu{_rep}_{g}_{m}") for m in range(T)]
                  if denom_mm:
                      acc_e = ppP.tile([1, 512], f32, tag="psP", name=f"Pdm{_rep}_{g}")
                  else:
                      acc_e = pio.tile([128, 512], f32, tag="acc_e", name=f"acc{_rep}_{g}")
                  e_prev = None
                  emit_transposes(8)  # head start for u(j=0..7)
                  for j in range(JT):
                      e = emit_scores(g, j, r_sb)
                      emit_transposes(j + 12)  # stay ~3 batches ahead of u
                      if e_prev is not None:
                          emit_u(g, j - 1, e_prev, Pu, acc_e)
                      e_prev = e
                  emit_u(g, JT - 1, e_prev, Pu, acc_e)
                  # next group's r + residual prefetch fills PE while norm chain runs
                  nxt = g + 1 if g + 1 < IG else (0 if _rep + 1 < repeat else None)
                  if nxt is not None:
                      nr_sb, nxrs = emit_r(nxt)
                  u_sb = emit_norm(g, Pu, acc_e)
                  emit_proj(g, u_sb, xrs)
                  if nxt is not None:
                      r_sb, xrs = nr_sb, nxrs

    nc.compile()
    return nc


def _host_inputs(x, gn_w, gn_b, wq, bq, wk, bk, wv, bv, wp, bp, mm_dtype_name=None):
    """Host-side weight fusion (fp64) + per-core input maps."""
    f32 = np.float32
    if mm_dtype_name is None:
        mm_dtype_name = MM_DTYPE
    if mm_dtype_name == "bf16":
        import ml_dtypes
        mmnp = ml_dtypes.bfloat16
    else:
        mmnp = np.float32
    wq64, wk64, wv64, wp64 = (np.asarray(w, np.float64) for w in (wq, wk, wv, wp))
    w1t = (wq64.T @ wk64).astype(f32)                      # [c'', c']
    w2t = (np.asarray(wp, np.float64) @ wv64).T.astype(f32)  # [c', c_out]
    rb = (wk64.T @ np.asarray(bq, np.float64)).astype(f32)   # [c']
    b2 = (wp64 @ np.asarray(bv, np.float64) + np.asarray(bp, np.float64)).astype(f32)

    def tile_vec(v):
        return np.ascontiguousarray(np.asarray(v, f32).reshape(T, 128).T)

    gs = C // NUM_GROUPS  # 16 channels per group; 8 local groups per 128-chan tile
    gi = np.zeros((128, 8), f32)
    git = np.zeros((8, 128), f32)
    for p in range(128):
        gi[p, p // gs] = 1.0 / gs  # group stat = mean of the 16 per-channel stats
        git[p // gs, p] = 1.0
    ident = np.eye(128, dtype=f32)

    cvec = np.ascontiguousarray(
        np.stack([tile_vec(rb), tile_vec(b2), tile_vec(gn_w), tile_vec(gn_b)], axis=1)
    )
    common = {
        "w1t": w1t.astype(mmnp),
        "w2t": np.ascontiguousarray(w2t).astype(mmnp),
        "cvec": cvec,
        "gi": gi,
        "git": git,
        "ident": ident.astype(mmnp),
        "onesc": np.ones((128, 1), np.float32),
        "onesr": np.ones((1, 128), np.float32),
    }

    x2 = np.asarray(x, f32).reshape(B, C, N)
    in_maps = []
    for core in range(8):
        b, s = divmod(core, 2)
        xb = x2[b]
        if s == 1:
            xb = np.concatenate([xb[:, NQ:], xb[:, :NQ]], axis=1)
        m = dict(common)
        m["xb"] = np.ascontiguousarray(xb)
        in_maps.append(m)
    return in_maps


def kernel(**inputs):
    global LAST_RESULTS
    from concourse.bass_utils import run_bass_kernel_spmd

    key = MM_DTYPE
    if key not in _PROGRAM_CACHE:
        _PROGRAM_CACHE[key] = _build_program(key)
    nc = _PROGRAM_CACHE[key]

    in_maps = _host_inputs(**{k: np.asarray(v) for k, v in inputs.items()})
    trace = bool(int(os.environ.get("BASS_KERNEL_TRACE", "0")))
    res = run_bass_kernel_spmd(
        nc, in_maps, list(range(8)), trace=trace,
        trace_cores=list(range(8)) if trace else None,
    )
    LAST_RESULTS = res

    out = np.empty((B, C, N), np.float32)
    for core in range(8):
        b, s = divmod(core, 2)
        out[b, :, NQ * s : NQ * (s + 1)] = res.results[core]["out"]
    return out.reshape(B, C, H, W)

